# revision 3
# baseline (speedup 1.0000x reference)
"""GCN message-passing kernel for 8 Trainium2 NeuronCores (v2).

Strategy (graph/data parallel):
  - Destination nodes sharded across 8 cores; within a core, nodes are
    packed into 49 blocks of 128 by a 2-D greedy packer that balances each
    block's in-edge counts split by source-table half, so per-block tile
    budgets (shared across cores, required by SPMD) carry ~3-6% padding
    instead of ~19%.
  - Per layer: hs = dinv * (x W^T + b) in bf16; the node table is
    AllGathered in TWO halves (blocks 0-24, then 25-48) so gathers from
    half 0 overlap the second AllGather.
  - Messages are fetched with batched indirect DMA gathers (dma_gather,
    KG 128-edge tiles per SWDGE call) from the DRAM tables and
    scatter-added per destination block with one-hot matmuls accumulated
    in PSUM. The one-hot S tiles are generated on-chip by a single DVE
    tensor_scalar is_equal of an iota row against per-edge dest ranks
    (pad slots carry rank 200 -> zero S row, so gathered garbage is
    ignored).
  - BN stats (sum, sumsq) via mask-vector matmuls, AllReduced across
    cores; BN apply + relu + residual on DVE/ACT.

kernel(**inputs) takes FULL inputs, returns the FULL output.
"""

import numpy as np

import concourse.bacc as bacc
import concourse.mybir as mybir
import concourse.tile as tile
from concourse.bass_utils import run_bass_kernel_spmd
from concourse.masks import make_identity

P = 128
F32 = mybir.dt.float32
BF16 = mybir.dt.bfloat16
AF = mybir.ActivationFunctionType
ALU = mybir.AluOpType

SENTINEL = 200.0  # rank value that never matches iota 0..127


class Cfg:
    def __init__(self, N, E, D, L, C, bpc=7, kg=8, bn_eps=1e-5):
        assert D == 128
        self.N, self.E, self.D, self.L, self.C = N, E, D, L, C
        self.NSH = N // C                      # real nodes per core (6250)
        assert self.NSH * C == N
        self.TPC = (self.NSH + P - 1) // P     # blocks per core (49)
        self.NPAD = self.TPC * P               # padded nodes per core (6272)
        self.H0B = 25                          # blocks in table half 0
        self.H1B = self.TPC - self.H0B         # 24
        self.H0 = self.H0B * P                 # 3200 rows
        self.H1 = self.H1B * P                 # 3072 rows
        self.T0ROWS = C * self.H0              # 25600
        self.T1ROWS = C * self.H1              # 24576
        assert self.T0ROWS < 32768 and self.T1ROWS < 32768
        self.KG = kg                           # 128-edge tiles per gather call
        self.BN_EPS = bn_eps


def _pack_half(d0, d1, nblocks, cap0, cap1):
    """Greedily pack len(d0) nodes into nblocks blocks of <=128 slots,
    keeping each block's (sum d0, sum d1) near-balanced. The last 3 blocks
    are 'loose' (higher cap). Returns block id per node."""
    n = len(d0)
    ntight = max(nblocks - 3, 0)
    order = np.argsort(-(d0 + d1), kind="stable")
    s0 = np.zeros(nblocks)
    s1 = np.zeros(nblocks)
    cnt = np.zeros(nblocks, np.int64)
    blk = np.empty(n, np.int64)
    for i in order:
        a, b = d0[i], d1[i]
        best, bestload = -1, 1e18
        for j in range(ntight):
            if cnt[j] >= P or s0[j] + a > cap0 or s1[j] + b > cap1:
                continue
            load = max((s0[j] + a) / cap0, (s1[j] + b) / cap1)
            if load < bestload:
                best, bestload = j, load
        if best < 0:
            # loose blocks (or overflow): pick min-loaded with room
            cands = [j for j in range(nblocks) if cnt[j] < P]
            best = min(cands, key=lambda j: s0[j] + s1[j])
        blk[i] = best
        s0[best] += a
        s1[best] += b
        cnt[best] += 1
    return blk, cnt


def _wrap16(idx):
    w = idx.reshape(-1, 16).T.astype(np.int16)
    return np.ascontiguousarray(np.tile(w, (8, 1)))


def _preprocess(cfg, x, edge_index, W, b, gamma, beta):
    N, C, NSH, NPAD, TPC = cfg.N, cfg.C, cfg.NSH, cfg.NPAD, cfg.TPC
    H0, H0B = cfg.H0, cfg.H0B
    row = np.asarray(edge_index[0], dtype=np.int64)
    col = np.asarray(edge_index[1], dtype=np.int64)
    x = np.asarray(x, dtype=np.float32)
    deg = np.bincount(row, minlength=N).astype(np.float32)

    loc = np.arange(N) % NSH
    half = (loc >= H0).astype(np.int64)        # source half per node (fixed)
    # per-node in-edge counts split by source half
    dest = col
    sh = half[row]
    cnt0 = np.bincount(dest[sh == 0], minlength=N)
    cnt1 = np.bincount(dest[sh == 1], minlength=N)

    # per-core packing: half-0 nodes -> blocks 0..H0B-1, half-1 -> rest
    newlocal = np.empty(N, np.int64)
    nvalid = np.zeros((C, TPC), np.int64)
    for c in range(C):
        base = c * NSH
        for h, (b0, nb) in enumerate(((0, H0B), (H0B, TPC - H0B))):
            ids = base + np.where((loc[base : base + NSH] >= H0) == bool(h))[0]
            d0 = cnt0[ids].astype(np.float64)
            d1 = cnt1[ids].astype(np.float64)
            avg0 = max(d0.sum() / nb, 1.0)
            avg1 = max(d1.sum() / nb, 1.0)
            cap0 = max(1024.0, np.ceil(avg0 / P) * P)
            cap1 = max(1024.0, np.ceil(avg1 / P) * P)
            blk, cnt = _pack_half(d0, d1, nb, cap0, cap1)
            rank = np.zeros(len(ids), np.int64)
            c2 = np.zeros(nb, np.int64)
            for k in range(len(ids)):
                rank[k] = c2[blk[k]]
                c2[blk[k]] += 1
            newlocal[ids] = (b0 + blk) * P + rank
            nvalid[c, b0 : b0 + nb] = c2
    assert nvalid.max() <= P

    # source table rows (per half)
    slot_of = newlocal  # node -> slot in its core's shard [0, NPAD)
    src_core = row // NSH
    src_slot = slot_of[row]
    src_half = (src_slot >= H0).astype(np.int64)
    src_t0 = src_core * cfg.H0 + src_slot
    src_t1 = src_core * cfg.H1 + (src_slot - H0)

    dst_core = col // NSH
    dst_slot = slot_of[col]
    dst_blk = dst_slot // P
    dst_rank = dst_slot % P

    # per (core, half): per-block edge lists sorted by src row
    cnts = np.zeros((C, 2, TPC), np.int64)
    per = {}
    for c in range(C):
        selc = dst_core == c
        for h in range(2):
            sel = selc & (src_half == h)
            srcs = (src_t0 if h == 0 else src_t1)[sel]
            blks = dst_blk[sel]
            rnks = dst_rank[sel]
            o = np.lexsort((srcs, blks))
            srcs, blks, rnks = srcs[o], blks[o], rnks[o]
            st = np.searchsorted(blks, np.arange(TPC))
            en = np.searchsorted(blks, np.arange(TPC) + 1)
            per[(c, h)] = (srcs, rnks, st, en)
            cnts[c, h] = en - st

    # shared tile budgets per (half, block)
    kb = np.maximum(1, -(-cnts.max(axis=0) // P))   # [2, TPC]
    NT = kb.sum(axis=1)                             # tiles per half stream
    off = np.zeros((2, TPC), np.int64)
    off[0, 1:] = np.cumsum(kb[0])[:-1]
    off[1, 1:] = np.cumsum(kb[1])[:-1]

    Wt = np.ascontiguousarray(np.transpose(np.asarray(W, np.float32), (0, 2, 1)))
    bT = np.ascontiguousarray(np.asarray(b, np.float32).T)
    iota = np.tile(np.arange(P, dtype=np.float32)[None, :], (P, 1))

    in_maps = []
    for c in range(C):
        idxs, ranks = [], []
        for h in range(2):
            idx = np.zeros(NT[h] * P, np.int64)
            rnk = np.full(NT[h] * P, SENTINEL, np.float32)
            srcs, rnks, st, en = per[(c, h)]
            for bidx in range(TPC):
                o0 = off[h, bidx] * P
                k = en[bidx] - st[bidx]
                idx[o0 : o0 + k] = srcs[st[bidx] : en[bidx]]
                rnk[o0 : o0 + k] = rnks[st[bidx] : en[bidx]]
            idxs.append(_wrap16(idx))
            ranks.append(np.ascontiguousarray(rnk.reshape(NT[h], P).T))
        ids = np.arange(c * NSH, (c + 1) * NSH)
        xin = np.zeros((NPAD, cfg.D), np.float32)
        xin[newlocal[ids]] = x[ids]
        degT = np.zeros((P, TPC), np.float32)
        degT[newlocal[ids] % P, newlocal[ids] // P] = deg[ids]
        maskv = (np.arange(P)[:, None] < nvalid[c][None, :]).astype(np.float32)
        in_maps.append(
            {
                "xin": xin,
                "wt": Wt,
                "bT": bT,
                "gamma": np.asarray(gamma, np.float32),
                "beta": np.asarray(beta, np.float32),
                "degT": degT,
                "maskv": maskv,
                "iota": iota,
                "idx0": idxs[0],
                "idx1": idxs[1],
                "rank0": ranks[0],
                "rank1": ranks[1],
            }
        )

    meta = dict(
        NT0=int(NT[0]),
        NT1=int(NT[1]),
        kb=kb,
        off=off,
        newlocal=newlocal,
    )
    return in_maps, meta


def _build(cfg, NT0, NT1, kb, off):
    N, D, L, C = cfg.N, cfg.D, cfg.L, cfg.C
    TPC, NPAD = cfg.TPC, cfg.NPAD
    KG = cfg.KG
    NTs = (NT0, NT1)

    nc = bacc.Bacc("TRN2", target_bir_lowering=False, debug=False, num_devices=C)

    xin = nc.dram_tensor("xin", [NPAD, D], F32, kind="ExternalInput")
    wt = nc.dram_tensor("wt", [L, D, D], F32, kind="ExternalInput")
    bT = nc.dram_tensor("bT", [D, L], F32, kind="ExternalInput")
    gamma_d = nc.dram_tensor("gamma", [L, D], F32, kind="ExternalInput")
    beta_d = nc.dram_tensor("beta", [L, D], F32, kind="ExternalInput")
    degT = nc.dram_tensor("degT", [P, TPC], F32, kind="ExternalInput")
    maskv_d = nc.dram_tensor("maskv", [P, TPC], F32, kind="ExternalInput")
    iota_d = nc.dram_tensor("iota", [P, P], F32, kind="ExternalInput")
    idx_d = [
        nc.dram_tensor(f"idx{h}", [P, NTs[h] * P // 16], mybir.dt.int16,
                       kind="ExternalInput")
        for h in range(2)
    ]
    rank_d = [
        nc.dram_tensor(f"rank{h}", [P, NTs[h]], F32, kind="ExternalInput")
        for h in range(2)
    ]
    out_d = nc.dram_tensor("out", [NPAD, D], F32, kind="ExternalOutput")

    rg = [list(range(C))]

    with tile.TileContext(nc) as tc:
        with (
            tc.tile_pool(name="persist", bufs=1) as pp,
            tc.tile_pool(name="msgp", bufs=10) as msgp,
            tc.tile_pool(name="bigtmp", bufs=1) as btp,
            tc.tile_pool(name="sp", bufs=12) as sp,
            tc.tile_pool(name="work", bufs=4) as wp,
            tc.tile_pool(name="psblk", bufs=3, space="PSUM") as psblk,
            tc.tile_pool(name="psmisc", bufs=2, space="PSUM") as psmisc,
            tc.tile_pool(name="psbc", bufs=1, space="PSUM") as psbc,
            tc.tile_pool(name="psstat", bufs=2, space="PSUM") as psstat,
            tc.tile_pool(name="dram", bufs=1, space="DRAM") as dp,
        ):
            # ---- persistent loads ----
            x_sb = pp.tile([P, TPC, D], F32)
            nc.sync.dma_start(x_sb[:], xin[:].rearrange("(t p) f -> p t f", p=P))
            wt_sb = pp.tile([P, L, D], F32)
            for l in range(L):
                nc.sync.dma_start(wt_sb[:, l, :], wt[l, :, :])
            bT_sb = pp.tile([P, L], F32)
            nc.sync.dma_start(bT_sb[:], bT[:])
            gb_sb = pp.tile([1, 2 * L, D], F32)
            for l in range(L):
                nc.sync.dma_start(gb_sb[:, l, :], gamma_d[l : l + 1, :])
                nc.sync.dma_start(gb_sb[:, L + l, :], beta_d[l : l + 1, :])
            deg_sb = pp.tile([P, TPC], F32)
            nc.sync.dma_start(deg_sb[:], degT[:])
            maskv_sb = pp.tile([P, TPC], F32)
            nc.sync.dma_start(maskv_sb[:], maskv_d[:])
            iota_sb = pp.tile([P, P], F32)
            nc.sync.dma_start(iota_sb[:], iota_d[:])
            idx_sb = [pp.tile([P, NTs[h] * P // 16], mybir.dt.int16) for h in range(2)]
            rank_sb = [pp.tile([P, NTs[h]], F32) for h in range(2)]
            for h in range(2):
                nc.sync.dma_start(idx_sb[h][:], idx_d[h][:])
                nc.sync.dma_start(rank_sb[h][:], rank_d[h][:])
            ident = pp.tile([P, P], F32)
            make_identity(nc, ident[:])
            ones1 = pp.tile([1, P], F32)
            nc.vector.memset(ones1[:], 1.0)

            # dinv = (deg > 0) / sqrt(max(deg, 1))
            dinv_sb = pp.tile([P, TPC], F32)
            t_a = wp.tile([P, TPC], F32, tag="dinv")
            nc.vector.tensor_scalar_max(t_a[:], deg_sb[:], 1.0)
            t_b = wp.tile([P, TPC], F32, tag="dinv")
            nc.vector.reciprocal(t_b[:], t_a[:])
            t_c = wp.tile([P, TPC], F32, tag="dinv")
            nc.scalar.sqrt(t_c[:], t_b[:])
            t_d = wp.tile([P, TPC], F32, tag="dinv")
            nc.vector.tensor_scalar(t_d[:], deg_sb[:], 0.0, None, ALU.is_gt)
            nc.vector.tensor_tensor(dinv_sb[:], t_c[:], t_d[:], ALU.mult)

            agg_sb = pp.tile([P, TPC, D], F32)
            hs_sb = pp.tile([P, TPC, D], BF16)

            # DRAM collective buffers
            sh_dr = [dp.tile([cfg.H0, D], BF16), dp.tile([cfg.H1, D], BF16)]
            tab_dr = [dp.tile([cfg.T0ROWS, D], BF16), dp.tile([cfg.T1ROWS, D], BF16)]
            stats_in = dp.tile([1, 2 * D], F32)
            stats_out = dp.tile([1, 2 * D], F32)

            for l in range(L):
                # ---- hs = dinv * (x @ W^T + b), bf16; AG in two halves ----
                for t in range(TPC):
                    xT_ps = psmisc.tile([P, P], F32, tag="ps")
                    nc.tensor.transpose(xT_ps[:], x_sb[:, t, :], ident[:])
                    xT = wp.tile([P, P], F32, tag="xT")
                    nc.vector.tensor_copy(xT[:], xT_ps[:])
                    hT_ps = psmisc.tile([P, P], F32, tag="ps")
                    nc.tensor.matmul(
                        out=hT_ps[:], lhsT=wt_sb[:, l, :], rhs=xT[:],
                        start=True, stop=True,
                    )
                    hb = wp.tile([P, P], F32, tag="hb")
                    nc.scalar.activation(
                        hb[:], hT_ps[:], AF.Identity, bias=bT_sb[:, l : l + 1]
                    )
                    h_rm_ps = psmisc.tile([P, P], F32, tag="ps")
                    nc.tensor.transpose(h_rm_ps[:], hb[:], ident[:])
                    nc.scalar.activation(
                        hs_sb[:, t, :], h_rm_ps[:], AF.Identity,
                        scale=dinv_sb[:, t : t + 1],
                    )
                    if t == cfg.H0B - 1:
                        nc.sync.dma_start(
                            sh_dr[0][:].rearrange("(t p) f -> p t f", p=P),
                            hs_sb[:, : cfg.H0B, :],
                        )
                        nc.gpsimd.collective_compute(
                            "AllGather", ALU.bypass,
                            ins=[sh_dr[0].opt()], outs=[tab_dr[0].opt()],
                            replica_groups=rg,
                        )
                nc.sync.dma_start(
                    sh_dr[1][:].rearrange("(t p) f -> p t f", p=P),
                    hs_sb[:, cfg.H0B :, :],
                )
                nc.gpsimd.collective_compute(
                    "AllGather", ALU.bypass,
                    ins=[sh_dr[1].opt()], outs=[tab_dr[1].opt()],
                    replica_groups=rg,
                )

                # ---- gather + one-hot matmul aggregation, half by half ----
                for h in range(2):
                    NT = NTs[h]
                    slot_of = {}
                    next_tile = 0

                    def _issue_gathers(upto, h=h, NT=NT):
                        nonlocal next_tile
                        while next_tile < min(upto, NT):
                            g0 = next_tile
                            g1 = min(g0 + KG, NT)
                            mt = msgp.tile([P, KG, D], BF16, tag="msg")
                            for i in range(g1 - g0):
                                slot_of[g0 + i] = (mt, i)
                            nc.gpsimd.dma_gather(
                                mt[:, : g1 - g0, :],
                                tab_dr[h][:],
                                idx_sb[h][:, g0 * 8 : g1 * 8],
                                (g1 - g0) * P, (g1 - g0) * P, D,
                            )
                            next_tile = g1

                    for bidx in range(TPC):
                        t0 = int(off[h, bidx])
                        nt = int(kb[h, bidx])
                        _issue_gathers(t0 + nt)
                        ps_b = psblk.tile([P, P], F32, tag="blk")
                        for j in range(nt):
                            s_t = sp.tile([P, P], BF16, tag="s")
                            nc.vector.tensor_scalar(
                                s_t[:], iota_sb[:],
                                rank_sb[h][:, t0 + j : t0 + j + 1],
                                None, ALU.is_equal,
                            )
                            mt, sl = slot_of[t0 + j]
                            nc.tensor.matmul(
                                out=ps_b[:], lhsT=s_t[:], rhs=mt[:, sl, :],
                                start=(j == 0), stop=(j == nt - 1),
                            )
                        if h == 0:
                            nc.scalar.activation(
                                agg_sb[:, bidx, :], ps_b[:], AF.Identity
                            )
                        else:
                            nc.vector.tensor_tensor(
                                agg_sb[:, bidx, :], agg_sb[:, bidx, :],
                                ps_b[:], ALU.add,
                            )

                # ---- dinv scale + BN stats pass ----
                stA_ps = psstat.tile([1, P], F32, tag="st")
                stB_ps = psstat.tile([1, P], F32, tag="st")
                for bidx in range(TPC):
                    nc.scalar.activation(
                        agg_sb[:, bidx, :], agg_sb[:, bidx, :], AF.Identity,
                        scale=dinv_sb[:, bidx : bidx + 1],
                    )
                    nc.tensor.matmul(
                        out=stA_ps[:],
                        lhsT=maskv_sb[:, bidx : bidx + 1],
                        rhs=agg_sb[:, bidx, :],
                        start=(bidx == 0), stop=(bidx == TPC - 1),
                        skip_group_check=True,
                    )
                    aggsq = wp.tile([P, P], F32, tag="aggsq")
                    nc.scalar.square(aggsq[:], agg_sb[:, bidx, :])
                    nc.tensor.matmul(
                        out=stB_ps[:],
                        lhsT=maskv_sb[:, bidx : bidx + 1],
                        rhs=aggsq[:],
                        start=(bidx == 0), stop=(bidx == TPC - 1),
                        skip_group_check=True,
                    )

                st_sb = wp.tile([1, 2, P], F32, tag="st")
                nc.vector.tensor_copy(st_sb[:, 0, :], stA_ps[:])
                nc.vector.tensor_copy(st_sb[:, 1, :], stB_ps[:])
                nc.sync.dma_start(stats_in[:], st_sb[:])
                nc.gpsimd.collective_compute(
                    "AllReduce", ALU.add,
                    ins=[stats_in.opt()], outs=[stats_out.opt()],
                    replica_groups=rg,
                )
                stg = wp.tile([1, 2, P], F32, tag="st")
                nc.sync.dma_start(stg[:], stats_out[:])

                # ---- scale/shift vectors on partition 0 ----
                vec = wp.tile([1, 8, P], F32, tag="vec")
                MU, MSQ, VAR, RSTD, SC, SH, T0, T1 = range(8)
                inv_n = 1.0 / float(N)
                nc.vector.tensor_scalar_mul(vec[:, MU, :], stg[:, 0, :], inv_n)
                nc.vector.tensor_scalar_mul(vec[:, MSQ, :], stg[:, 1, :], inv_n)
                nc.vector.tensor_tensor(
                    vec[:, T0, :], vec[:, MU, :], vec[:, MU, :], ALU.mult
                )
                nc.vector.tensor_tensor(
                    vec[:, VAR, :], vec[:, MSQ, :], vec[:, T0, :], ALU.subtract
                )
                nc.vector.tensor_scalar_add(vec[:, T1, :], vec[:, VAR, :], cfg.BN_EPS)
                nc.vector.reciprocal(vec[:, T0, :], vec[:, T1, :])
                nc.scalar.sqrt(vec[:, RSTD, :], vec[:, T0, :])
                nc.vector.tensor_tensor(
                    vec[:, SC, :], gb_sb[:, l, :], vec[:, RSTD, :], ALU.mult
                )
                nc.vector.tensor_tensor(
                    vec[:, T0, :], vec[:, MU, :], vec[:, SC, :], ALU.mult
                )
                nc.vector.tensor_tensor(
                    vec[:, SH, :], gb_sb[:, L + l, :], vec[:, T0, :], ALU.subtract
                )
                bc_ps = psbc.tile([P, 2 * P], F32, tag="bc")
                nc.tensor.matmul(
                    out=bc_ps[:], lhsT=ones1[:], rhs=vec[:, SC : SH + 1, :],
                    start=True, stop=True,
                )
                screp = wp.tile([P, 2, P], F32, tag="screp")
                nc.vector.tensor_copy(screp[:], bc_ps[:])

                # ---- BN apply + relu + residual ----
                t1 = btp.tile([P, TPC, D], F32, tag="t1")
                nc.vector.tensor_tensor(
                    t1[:], agg_sb[:],
                    screp[:, 0:1, :].to_broadcast([P, TPC, D]), ALU.mult,
                )
                nc.vector.tensor_tensor(
                    t1[:], t1[:],
                    screp[:, 1:2, :].to_broadcast([P, TPC, D]), ALU.add,
                )
                nc.scalar.activation(t1[:], t1[:], AF.Relu)
                nc.vector.tensor_tensor(x_sb[:], x_sb[:], t1[:], ALU.add)

            nc.sync.dma_start(out_d[:].rearrange("(t p) f -> p t f", p=P), x_sb[:])

    nc.compile()
    return nc


_CACHE = {}


def _get_nc(cfg, NT0, NT1, kb, off):
    key = (cfg.N, cfg.E, cfg.L, cfg.C, cfg.KG, NT0, NT1,
           tuple(kb.ravel()), tuple(off.ravel()))
    if key not in _CACHE:
        _CACHE[key] = _build(cfg, NT0, NT1, kb, off)
    return _CACHE[key]


def run(cfg, inputs, trace=False):
    in_maps, meta = _preprocess(cfg, **inputs)
    nc = _get_nc(cfg, meta["NT0"], meta["NT1"], meta["kb"], meta["off"])
    res = run_bass_kernel_spmd(nc, in_maps, core_ids=list(range(cfg.C)), trace=trace)
    newlocal = meta["newlocal"]
    xfull = np.empty((cfg.N, cfg.D), np.float32)
    for c in range(cfg.C):
        ids = np.arange(c * cfg.NSH, (c + 1) * cfg.NSH)
        xfull[ids] = res.results[c]["out"][newlocal[ids]]
    return xfull, res


def kernel(x, edge_index, W, b, gamma, beta):
    cfg = Cfg(N=50000, E=800000, D=128, L=3, C=8, kg=8)
    out, _ = run(
        cfg, dict(x=x, edge_index=edge_index, W=W, b=b, gamma=gamma, beta=beta)
    )
    return out


# revision 5
# speedup vs baseline: 1.0743x; 1.0743x over previous
"""GCN message-passing kernel for 8 Trainium2 NeuronCores (v2).

Strategy (graph/data parallel):
  - Destination nodes sharded across 8 cores; within a core, nodes are
    packed into 49 blocks of 128 by a 2-D greedy packer that balances each
    block's in-edge counts split by source-table half, so per-block tile
    budgets (shared across cores, required by SPMD) carry ~3-6% padding
    instead of ~19%.
  - Per layer: hs = dinv * (x W^T + b) in bf16; the node table is
    AllGathered in TWO halves (blocks 0-24, then 25-48) so gathers from
    half 0 overlap the second AllGather.
  - Messages are fetched with batched indirect DMA gathers (dma_gather,
    KG 128-edge tiles per SWDGE call) from the DRAM tables and
    scatter-added per destination block with one-hot matmuls accumulated
    in PSUM. The one-hot S tiles are generated on-chip by a single DVE
    tensor_scalar is_equal of an iota row against per-edge dest ranks
    (pad slots carry rank 200 -> zero S row, so gathered garbage is
    ignored).
  - BN stats (sum, sumsq) via mask-vector matmuls, AllReduced across
    cores; BN apply + relu + residual on DVE/ACT.

kernel(**inputs) takes FULL inputs, returns the FULL output.
"""

import numpy as np

import concourse.bacc as bacc
import concourse.mybir as mybir
import concourse.tile as tile
from concourse.bass_utils import run_bass_kernel_spmd
from concourse.masks import make_identity

P = 128
F32 = mybir.dt.float32
BF16 = mybir.dt.bfloat16
AF = mybir.ActivationFunctionType
ALU = mybir.AluOpType

SENTINEL = 200.0  # rank value that never matches iota 0..127


class Cfg:
    def __init__(self, N, E, D, L, C, bpc=7, kg=8, bn_eps=1e-5):
        assert D == 128
        self.N, self.E, self.D, self.L, self.C = N, E, D, L, C
        self.NSH = N // C                      # real nodes per core (6250)
        assert self.NSH * C == N
        self.TPC = (self.NSH + P - 1) // P     # blocks per core (49)
        self.NPAD = self.TPC * P               # padded nodes per core (6272)
        self.H0B = 25                          # blocks in table half 0
        self.H1B = self.TPC - self.H0B         # 24
        self.H0 = self.H0B * P                 # 3200 rows
        self.H1 = self.H1B * P                 # 3072 rows
        self.T0ROWS = C * self.H0              # 25600
        self.T1ROWS = C * self.H1              # 24576
        assert self.T0ROWS < 32768 and self.T1ROWS < 32768
        self.KG = kg                           # 128-edge tiles per gather call
        self.BN_EPS = bn_eps


def _pack_half(d0, d1, nblocks, cap0, cap1):
    """Greedily pack len(d0) nodes into nblocks blocks of <=128 slots,
    keeping each block's (sum d0, sum d1) near-balanced. The last 3 blocks
    are 'loose' (higher cap). Returns block id per node."""
    n = len(d0)
    ntight = max(nblocks - 3, 0)
    order = np.argsort(-(d0 + d1), kind="stable")
    s0 = np.zeros(nblocks)
    s1 = np.zeros(nblocks)
    cnt = np.zeros(nblocks, np.int64)
    blk = np.empty(n, np.int64)
    for i in order:
        a, b = d0[i], d1[i]
        best, bestload = -1, 1e18
        for j in range(ntight):
            if cnt[j] >= P or s0[j] + a > cap0 or s1[j] + b > cap1:
                continue
            load = max((s0[j] + a) / cap0, (s1[j] + b) / cap1)
            if load < bestload:
                best, bestload = j, load
        if best < 0:
            # loose blocks (or overflow): pick min-loaded with room
            cands = [j for j in range(nblocks) if cnt[j] < P]
            best = min(cands, key=lambda j: s0[j] + s1[j])
        blk[i] = best
        s0[best] += a
        s1[best] += b
        cnt[best] += 1
    return blk, cnt


def _wrap16(idx):
    w = idx.reshape(-1, 16).T.astype(np.int16)
    return np.ascontiguousarray(np.tile(w, (8, 1)))


def _preprocess(cfg, x, edge_index, W, b, gamma, beta):
    N, C, NSH, NPAD, TPC = cfg.N, cfg.C, cfg.NSH, cfg.NPAD, cfg.TPC
    H0, H0B = cfg.H0, cfg.H0B
    row = np.asarray(edge_index[0], dtype=np.int64)
    col = np.asarray(edge_index[1], dtype=np.int64)
    x = np.asarray(x, dtype=np.float32)
    deg = np.bincount(row, minlength=N).astype(np.float32)

    loc = np.arange(N) % NSH
    half = (loc >= H0).astype(np.int64)        # source half per node (fixed)
    # per-node in-edge counts split by source half
    dest = col
    sh = half[row]
    cnt0 = np.bincount(dest[sh == 0], minlength=N)
    cnt1 = np.bincount(dest[sh == 1], minlength=N)

    # per-core packing: half-0 nodes -> blocks 0..H0B-1, half-1 -> rest
    newlocal = np.empty(N, np.int64)
    nvalid = np.zeros((C, TPC), np.int64)
    for c in range(C):
        base = c * NSH
        for h, (b0, nb) in enumerate(((0, H0B), (H0B, TPC - H0B))):
            ids = base + np.where((loc[base : base + NSH] >= H0) == bool(h))[0]
            d0 = cnt0[ids].astype(np.float64)
            d1 = cnt1[ids].astype(np.float64)
            avg0 = max(d0.sum() / nb, 1.0)
            avg1 = max(d1.sum() / nb, 1.0)
            cap0 = max(1024.0, np.ceil(avg0 / P) * P)
            cap1 = max(1024.0, np.ceil(avg1 / P) * P)
            blk, cnt = _pack_half(d0, d1, nb, cap0, cap1)
            rank = np.zeros(len(ids), np.int64)
            c2 = np.zeros(nb, np.int64)
            for k in range(len(ids)):
                rank[k] = c2[blk[k]]
                c2[blk[k]] += 1
            newlocal[ids] = (b0 + blk) * P + rank
            nvalid[c, b0 : b0 + nb] = c2
    assert nvalid.max() <= P

    # source table rows (per half)
    slot_of = newlocal  # node -> slot in its core's shard [0, NPAD)
    src_core = row // NSH
    src_slot = slot_of[row]
    src_half = (src_slot >= H0).astype(np.int64)
    src_t0 = src_core * cfg.H0 + src_slot
    src_t1 = src_core * cfg.H1 + (src_slot - H0)

    dst_core = col // NSH
    dst_slot = slot_of[col]
    dst_blk = dst_slot // P
    dst_rank = dst_slot % P

    # per (core, half): per-block edge lists sorted by src row
    cnts = np.zeros((C, 2, TPC), np.int64)
    per = {}
    for c in range(C):
        selc = dst_core == c
        for h in range(2):
            sel = selc & (src_half == h)
            srcs = (src_t0 if h == 0 else src_t1)[sel]
            blks = dst_blk[sel]
            rnks = dst_rank[sel]
            o = np.lexsort((srcs, blks))
            srcs, blks, rnks = srcs[o], blks[o], rnks[o]
            st = np.searchsorted(blks, np.arange(TPC))
            en = np.searchsorted(blks, np.arange(TPC) + 1)
            per[(c, h)] = (srcs, rnks, st, en)
            cnts[c, h] = en - st

    # shared tile budgets per (half, block)
    kb = np.maximum(1, -(-cnts.max(axis=0) // P))   # [2, TPC]
    NT = kb.sum(axis=1)                             # tiles per half stream
    off = np.zeros((2, TPC), np.int64)
    off[0, 1:] = np.cumsum(kb[0])[:-1]
    off[1, 1:] = np.cumsum(kb[1])[:-1]

    Wt = np.ascontiguousarray(np.transpose(np.asarray(W, np.float32), (0, 2, 1)))
    bT = np.ascontiguousarray(np.asarray(b, np.float32).T)
    iota = np.tile(np.arange(P, dtype=np.float32)[None, :], (P, 1))

    in_maps = []
    for c in range(C):
        idxs, ranks = [], []
        for h in range(2):
            idx = np.zeros(NT[h] * P, np.int64)
            rnk = np.full(NT[h] * P, SENTINEL, np.float32)
            srcs, rnks, st, en = per[(c, h)]
            for bidx in range(TPC):
                o0 = off[h, bidx] * P
                k = en[bidx] - st[bidx]
                idx[o0 : o0 + k] = srcs[st[bidx] : en[bidx]]
                rnk[o0 : o0 + k] = rnks[st[bidx] : en[bidx]]
            idxs.append(_wrap16(idx))
            ranks.append(np.ascontiguousarray(rnk.reshape(NT[h], P).T))
        ids = np.arange(c * NSH, (c + 1) * NSH)
        xin = np.zeros((NPAD, cfg.D), np.float32)
        xin[newlocal[ids]] = x[ids]
        degT = np.zeros((P, TPC), np.float32)
        degT[newlocal[ids] % P, newlocal[ids] // P] = deg[ids]
        maskv = (np.arange(P)[:, None] < nvalid[c][None, :]).astype(np.float32)
        in_maps.append(
            {
                "xin": xin,
                "wt": Wt,
                "bT": bT,
                "gamma": np.asarray(gamma, np.float32),
                "beta": np.asarray(beta, np.float32),
                "degT": degT,
                "maskv": maskv,
                "iota": iota,
                "idx0": idxs[0],
                "idx1": idxs[1],
                "rank0": ranks[0],
                "rank1": ranks[1],
            }
        )

    meta = dict(
        NT0=int(NT[0]),
        NT1=int(NT[1]),
        kb=kb,
        off=off,
        newlocal=newlocal,
    )
    return in_maps, meta


def _build(cfg, NT0, NT1, kb, off):
    N, D, L, C = cfg.N, cfg.D, cfg.L, cfg.C
    TPC, NPAD = cfg.TPC, cfg.NPAD
    KG = cfg.KG
    NTs = (NT0, NT1)

    nc = bacc.Bacc("TRN2", target_bir_lowering=False, debug=False, num_devices=C)

    xin = nc.dram_tensor("xin", [NPAD, D], F32, kind="ExternalInput")
    wt = nc.dram_tensor("wt", [L, D, D], F32, kind="ExternalInput")
    bT = nc.dram_tensor("bT", [D, L], F32, kind="ExternalInput")
    gamma_d = nc.dram_tensor("gamma", [L, D], F32, kind="ExternalInput")
    beta_d = nc.dram_tensor("beta", [L, D], F32, kind="ExternalInput")
    degT = nc.dram_tensor("degT", [P, TPC], F32, kind="ExternalInput")
    maskv_d = nc.dram_tensor("maskv", [P, TPC], F32, kind="ExternalInput")
    iota_d = nc.dram_tensor("iota", [P, P], F32, kind="ExternalInput")
    idx_d = [
        nc.dram_tensor(f"idx{h}", [P, NTs[h] * P // 16], mybir.dt.int16,
                       kind="ExternalInput")
        for h in range(2)
    ]
    rank_d = [
        nc.dram_tensor(f"rank{h}", [P, NTs[h]], F32, kind="ExternalInput")
        for h in range(2)
    ]
    out_d = nc.dram_tensor("out", [NPAD, D], F32, kind="ExternalOutput")

    rg = [list(range(C))]

    with tile.TileContext(nc) as tc:
        with (
            tc.tile_pool(name="persist", bufs=1) as pp,
            tc.tile_pool(name="msgp", bufs=10) as msgp,
            tc.tile_pool(name="bigtmp", bufs=1) as btp,
            tc.tile_pool(name="sp", bufs=12) as sp,
            tc.tile_pool(name="work", bufs=4) as wp,
            tc.tile_pool(name="psblk", bufs=3, space="PSUM") as psblk,
            tc.tile_pool(name="psmisc", bufs=2, space="PSUM") as psmisc,
            tc.tile_pool(name="psbc", bufs=1, space="PSUM") as psbc,
            tc.tile_pool(name="psstat", bufs=2, space="PSUM") as psstat,
            tc.tile_pool(name="dram", bufs=1, space="DRAM") as dp,
        ):
            # ---- persistent loads ----
            x_sb = pp.tile([P, TPC, D], F32)
            nc.sync.dma_start(x_sb[:], xin[:].rearrange("(t p) f -> p t f", p=P))
            wt_sb = pp.tile([P, L, D], F32)
            for l in range(L):
                nc.sync.dma_start(wt_sb[:, l, :], wt[l, :, :])
            bT_sb = pp.tile([P, L], F32)
            nc.sync.dma_start(bT_sb[:], bT[:])
            gb_sb = pp.tile([1, 2 * L, D], F32)
            for l in range(L):
                nc.sync.dma_start(gb_sb[:, l, :], gamma_d[l : l + 1, :])
                nc.sync.dma_start(gb_sb[:, L + l, :], beta_d[l : l + 1, :])
            deg_sb = pp.tile([P, TPC], F32)
            nc.sync.dma_start(deg_sb[:], degT[:])
            maskv_sb = pp.tile([P, TPC], F32)
            nc.sync.dma_start(maskv_sb[:], maskv_d[:])
            iota_sb = pp.tile([P, P], F32)
            nc.sync.dma_start(iota_sb[:], iota_d[:])
            idx0_sb = pp.tile([P, NTs[0] * P // 16], mybir.dt.int16)
            idx1_sb = pp.tile([P, NTs[1] * P // 16], mybir.dt.int16)
            rank0_sb = pp.tile([P, NTs[0]], F32)
            rank1_sb = pp.tile([P, NTs[1]], F32)
            idx_sb = [idx0_sb, idx1_sb]
            rank_sb = [rank0_sb, rank1_sb]
            for h in range(2):
                nc.sync.dma_start(idx_sb[h][:], idx_d[h][:])
                nc.sync.dma_start(rank_sb[h][:], rank_d[h][:])
            ident = pp.tile([P, P], F32)
            make_identity(nc, ident[:])
            ones1 = pp.tile([1, P], F32)
            nc.vector.memset(ones1[:], 1.0)

            # dinv = (deg > 0) / sqrt(max(deg, 1))
            dinv_sb = pp.tile([P, TPC], F32)
            t_a = wp.tile([P, TPC], F32, tag="dinv")
            nc.vector.tensor_scalar_max(t_a[:], deg_sb[:], 1.0)
            t_b = wp.tile([P, TPC], F32, tag="dinv")
            nc.vector.reciprocal(t_b[:], t_a[:])
            t_c = wp.tile([P, TPC], F32, tag="dinv")
            nc.scalar.sqrt(t_c[:], t_b[:])
            t_d = wp.tile([P, TPC], F32, tag="dinv")
            nc.vector.tensor_scalar(t_d[:], deg_sb[:], 0.0, None, ALU.is_gt)
            nc.vector.tensor_tensor(dinv_sb[:], t_c[:], t_d[:], ALU.mult)

            agg_sb = pp.tile([P, TPC, D], F32)
            hs_sb = pp.tile([P, TPC, D], BF16)

            # DRAM collective buffers
            sh0_dr = dp.tile([cfg.H0, D], BF16)
            sh1_dr = dp.tile([cfg.H1, D], BF16)
            tab0_dr = dp.tile([cfg.T0ROWS, D], BF16)
            tab1_dr = dp.tile([cfg.T1ROWS, D], BF16)
            sh_dr = [sh0_dr, sh1_dr]
            tab_dr = [tab0_dr, tab1_dr]
            stats_in = dp.tile([1, 2 * D], F32)
            stats_out = dp.tile([1, 2 * D], F32)

            for l in range(L):
                # ---- hs = dinv * (x @ W^T + b), bf16; AG in two halves ----
                for t in range(TPC):
                    xT_ps = psmisc.tile([P, P], F32, tag="ps")
                    nc.tensor.transpose(xT_ps[:], x_sb[:, t, :], ident[:])
                    xT = wp.tile([P, P], F32, tag="xT")
                    nc.vector.tensor_copy(xT[:], xT_ps[:])
                    hT_ps = psmisc.tile([P, P], F32, tag="ps")
                    nc.tensor.matmul(
                        out=hT_ps[:], lhsT=wt_sb[:, l, :], rhs=xT[:],
                        start=True, stop=True,
                    )
                    hb = wp.tile([P, P], F32, tag="hb")
                    nc.scalar.activation(
                        hb[:], hT_ps[:], AF.Identity, bias=bT_sb[:, l : l + 1]
                    )
                    h_rm_ps = psmisc.tile([P, P], F32, tag="ps")
                    nc.tensor.transpose(h_rm_ps[:], hb[:], ident[:])
                    nc.scalar.activation(
                        hs_sb[:, t, :], h_rm_ps[:], AF.Identity,
                        scale=dinv_sb[:, t : t + 1],
                    )
                    if t == cfg.H0B - 1:
                        nc.sync.dma_start(
                            sh_dr[0][:].rearrange("(t p) f -> p t f", p=P),
                            hs_sb[:, : cfg.H0B, :],
                        )
                        nc.gpsimd.collective_compute(
                            "AllGather", ALU.bypass,
                            ins=[sh_dr[0].opt()], outs=[tab_dr[0].opt()],
                            replica_groups=rg,
                        )
                nc.sync.dma_start(
                    sh_dr[1][:].rearrange("(t p) f -> p t f", p=P),
                    hs_sb[:, cfg.H0B :, :],
                )
                nc.gpsimd.collective_compute(
                    "AllGather", ALU.bypass,
                    ins=[sh_dr[1].opt()], outs=[tab_dr[1].opt()],
                    replica_groups=rg,
                )

                # ---- gather + one-hot matmul aggregation, half by half ----
                for h in range(2):
                    NT = NTs[h]
                    slot_of = {}
                    next_tile = 0

                    def _issue_gathers(upto, h=h, NT=NT):
                        nonlocal next_tile
                        while next_tile < min(upto, NT):
                            g0 = next_tile
                            g1 = min(g0 + KG, NT)
                            mt = msgp.tile([P, KG, D], BF16, tag="msg")
                            for i in range(g1 - g0):
                                slot_of[g0 + i] = (mt, i)
                            nc.gpsimd.dma_gather(
                                mt[:, : g1 - g0, :],
                                tab_dr[h][:],
                                idx_sb[h][:, g0 * 8 : g1 * 8],
                                (g1 - g0) * P, (g1 - g0) * P, D,
                            )
                            next_tile = g1

                    for bidx in range(TPC):
                        t0 = int(off[h, bidx])
                        nt = int(kb[h, bidx])
                        _issue_gathers(t0 + nt)
                        ps_b = psblk.tile([P, P], F32, tag="blk")
                        for j in range(nt):
                            s_t = sp.tile([P, P], BF16, tag="s")
                            nc.vector.tensor_scalar(
                                s_t[:], iota_sb[:],
                                rank_sb[h][:, t0 + j : t0 + j + 1],
                                None, ALU.is_equal,
                            )
                            mt, sl = slot_of[t0 + j]
                            nc.tensor.matmul(
                                out=ps_b[:], lhsT=s_t[:], rhs=mt[:, sl, :],
                                start=(j == 0), stop=(j == nt - 1),
                            )
                        if h == 0:
                            nc.scalar.activation(
                                agg_sb[:, bidx, :], ps_b[:], AF.Identity
                            )
                        else:
                            nc.vector.tensor_tensor(
                                agg_sb[:, bidx, :], agg_sb[:, bidx, :],
                                ps_b[:], ALU.add,
                            )

                # ---- dinv scale + BN stats pass ----
                stA_ps = psstat.tile([1, P], F32, tag="st")
                stB_ps = psstat.tile([1, P], F32, tag="st")
                for bidx in range(TPC):
                    nc.scalar.activation(
                        agg_sb[:, bidx, :], agg_sb[:, bidx, :], AF.Identity,
                        scale=dinv_sb[:, bidx : bidx + 1],
                    )
                    nc.tensor.matmul(
                        out=stA_ps[:],
                        lhsT=maskv_sb[:, bidx : bidx + 1],
                        rhs=agg_sb[:, bidx, :],
                        start=(bidx == 0), stop=(bidx == TPC - 1),
                        skip_group_check=True,
                    )
                    aggsq = wp.tile([P, P], F32, tag="aggsq")
                    nc.scalar.square(aggsq[:], agg_sb[:, bidx, :])
                    nc.tensor.matmul(
                        out=stB_ps[:],
                        lhsT=maskv_sb[:, bidx : bidx + 1],
                        rhs=aggsq[:],
                        start=(bidx == 0), stop=(bidx == TPC - 1),
                        skip_group_check=True,
                    )

                st_sb = wp.tile([1, 2, P], F32, tag="st")
                nc.vector.tensor_copy(st_sb[:, 0, :], stA_ps[:])
                nc.vector.tensor_copy(st_sb[:, 1, :], stB_ps[:])
                nc.sync.dma_start(stats_in[:], st_sb[:])
                nc.gpsimd.collective_compute(
                    "AllReduce", ALU.add,
                    ins=[stats_in.opt()], outs=[stats_out.opt()],
                    replica_groups=rg,
                )
                stg = wp.tile([1, 2, P], F32, tag="st")
                nc.sync.dma_start(stg[:], stats_out[:])

                # ---- scale/shift vectors on partition 0 ----
                vec = wp.tile([1, 8, P], F32, tag="vec")
                MU, MSQ, VAR, RSTD, SC, SH, T0, T1 = range(8)
                inv_n = 1.0 / float(N)
                nc.vector.tensor_scalar_mul(vec[:, MU, :], stg[:, 0, :], inv_n)
                nc.vector.tensor_scalar_mul(vec[:, MSQ, :], stg[:, 1, :], inv_n)
                nc.vector.tensor_tensor(
                    vec[:, T0, :], vec[:, MU, :], vec[:, MU, :], ALU.mult
                )
                nc.vector.tensor_tensor(
                    vec[:, VAR, :], vec[:, MSQ, :], vec[:, T0, :], ALU.subtract
                )
                nc.vector.tensor_scalar_add(vec[:, T1, :], vec[:, VAR, :], cfg.BN_EPS)
                nc.vector.reciprocal(vec[:, T0, :], vec[:, T1, :])
                nc.scalar.sqrt(vec[:, RSTD, :], vec[:, T0, :])
                nc.vector.tensor_tensor(
                    vec[:, SC, :], gb_sb[:, l, :], vec[:, RSTD, :], ALU.mult
                )
                nc.vector.tensor_tensor(
                    vec[:, T0, :], vec[:, MU, :], vec[:, SC, :], ALU.mult
                )
                nc.vector.tensor_tensor(
                    vec[:, SH, :], gb_sb[:, L + l, :], vec[:, T0, :], ALU.subtract
                )
                bc_ps = psbc.tile([P, 2 * P], F32, tag="bc")
                nc.tensor.matmul(
                    out=bc_ps[:], lhsT=ones1[:], rhs=vec[:, SC : SH + 1, :],
                    start=True, stop=True,
                )
                screp = wp.tile([P, 2, P], F32, tag="screp")
                nc.vector.tensor_copy(screp[:], bc_ps[:])

                # ---- BN apply + relu + residual ----
                t1 = btp.tile([P, TPC, D], F32, tag="t1")
                nc.vector.tensor_tensor(
                    t1[:], agg_sb[:],
                    screp[:, 0:1, :].to_broadcast([P, TPC, D]), ALU.mult,
                )
                nc.vector.tensor_tensor(
                    t1[:], t1[:],
                    screp[:, 1:2, :].to_broadcast([P, TPC, D]), ALU.add,
                )
                nc.scalar.activation(t1[:], t1[:], AF.Relu)
                nc.vector.tensor_tensor(x_sb[:], x_sb[:], t1[:], ALU.add)

            nc.sync.dma_start(out_d[:].rearrange("(t p) f -> p t f", p=P), x_sb[:])

    nc.compile()
    return nc


_CACHE = {}


def _get_nc(cfg, NT0, NT1, kb, off):
    key = (cfg.N, cfg.E, cfg.L, cfg.C, cfg.KG, NT0, NT1,
           tuple(kb.ravel()), tuple(off.ravel()))
    if key not in _CACHE:
        _CACHE[key] = _build(cfg, NT0, NT1, kb, off)
    return _CACHE[key]


def run(cfg, inputs, trace=False):
    in_maps, meta = _preprocess(cfg, **inputs)
    nc = _get_nc(cfg, meta["NT0"], meta["NT1"], meta["kb"], meta["off"])
    res = run_bass_kernel_spmd(nc, in_maps, core_ids=list(range(cfg.C)), trace=trace)
    newlocal = meta["newlocal"]
    xfull = np.empty((cfg.N, cfg.D), np.float32)
    for c in range(cfg.C):
        ids = np.arange(c * cfg.NSH, (c + 1) * cfg.NSH)
        xfull[ids] = res.results[c]["out"][newlocal[ids]]
    return xfull, res


def kernel(x, edge_index, W, b, gamma, beta):
    cfg = Cfg(N=50000, E=800000, D=128, L=3, C=8, kg=8)
    out, _ = run(
        cfg, dict(x=x, edge_index=edge_index, W=W, b=b, gamma=gamma, beta=beta)
    )
    return out


# revision 12
# speedup vs baseline: 1.0797x; 1.0050x over previous
"""GCN message-passing kernel for 8 Trainium2 NeuronCores (v2).

Strategy (graph/data parallel):
  - Destination nodes sharded across 8 cores; within a core, nodes are
    packed into 49 blocks of 128 by a 2-D greedy packer that balances each
    block's in-edge counts split by source-table half, so per-block tile
    budgets (shared across cores, required by SPMD) carry ~3-6% padding
    instead of ~19%.
  - Per layer: hs = dinv * (x W^T + b) in bf16; the node table is
    AllGathered in TWO halves (blocks 0-24, then 25-48) so gathers from
    half 0 overlap the second AllGather.
  - Messages are fetched with batched indirect DMA gathers (dma_gather,
    KG 128-edge tiles per SWDGE call) from the DRAM tables and
    scatter-added per destination block with one-hot matmuls accumulated
    in PSUM. The one-hot S tiles are generated on-chip by a single DVE
    tensor_scalar is_equal of an iota row against per-edge dest ranks
    (pad slots carry rank 200 -> zero S row, so gathered garbage is
    ignored).
  - BN stats (sum, sumsq) via mask-vector matmuls, AllReduced across
    cores; BN apply + relu + residual on DVE/ACT.

kernel(**inputs) takes FULL inputs, returns the FULL output.
"""

import numpy as np

import concourse.bacc as bacc
import concourse.mybir as mybir
import concourse.tile as tile
from concourse.bass_utils import run_bass_kernel_spmd
from concourse.masks import make_identity

P = 128
F32 = mybir.dt.float32
BF16 = mybir.dt.bfloat16
AF = mybir.ActivationFunctionType
ALU = mybir.AluOpType

SENTINEL = 200.0  # rank value that never matches iota 0..127


class Cfg:
    def __init__(self, N, E, D, L, C, bpc=7, kg=8, bn_eps=1e-5):
        assert D == 128
        self.N, self.E, self.D, self.L, self.C = N, E, D, L, C
        self.NSH = N // C                      # real nodes per core (6250)
        assert self.NSH * C == N
        self.TPC = (self.NSH + P - 1) // P     # blocks per core (49)
        self.NPAD = self.TPC * P               # padded nodes per core (6272)
        self.H0B = 25                          # blocks in table half 0
        self.H1B = self.TPC - self.H0B         # 24
        self.H0 = self.H0B * P                 # 3200 rows
        self.H1 = self.H1B * P                 # 3072 rows
        self.T0ROWS = C * self.H0              # 25600
        self.T1ROWS = C * self.H1              # 24576
        assert self.T0ROWS < 32768 and self.T1ROWS < 32768
        self.KG = kg                           # 128-edge tiles per gather call
        self.BN_EPS = bn_eps


def _pack_half(d0, d1, nblocks, cap0, cap1):
    """Greedily pack len(d0) nodes into nblocks blocks of <=128 slots,
    keeping each block's (sum d0, sum d1) near-balanced. The last 3 blocks
    are 'loose' (higher cap). Returns block id per node."""
    n = len(d0)
    ntight = max(nblocks - 3, 0)
    order = np.argsort(-(d0 + d1), kind="stable")
    s0 = np.zeros(nblocks)
    s1 = np.zeros(nblocks)
    cnt = np.zeros(nblocks, np.int64)
    blk = np.empty(n, np.int64)
    for i in order:
        a, b = d0[i], d1[i]
        best, bestload = -1, 1e18
        for j in range(ntight):
            if cnt[j] >= P or s0[j] + a > cap0 or s1[j] + b > cap1:
                continue
            load = max((s0[j] + a) / cap0, (s1[j] + b) / cap1)
            if load < bestload:
                best, bestload = j, load
        if best < 0:
            # loose blocks (or overflow): pick min-loaded with room
            cands = [j for j in range(nblocks) if cnt[j] < P]
            best = min(cands, key=lambda j: s0[j] + s1[j])
        blk[i] = best
        s0[best] += a
        s1[best] += b
        cnt[best] += 1
    return blk, cnt


def _wrap16(idx):
    w = idx.reshape(-1, 16).T.astype(np.int16)
    return np.ascontiguousarray(np.tile(w, (8, 1)))


def _preprocess(cfg, x, edge_index, W, b, gamma, beta):
    N, C, NSH, NPAD, TPC = cfg.N, cfg.C, cfg.NSH, cfg.NPAD, cfg.TPC
    H0, H0B = cfg.H0, cfg.H0B
    row = np.asarray(edge_index[0], dtype=np.int64)
    col = np.asarray(edge_index[1], dtype=np.int64)
    x = np.asarray(x, dtype=np.float32)
    deg = np.bincount(row, minlength=N).astype(np.float32)

    loc = np.arange(N) % NSH
    half = (loc >= H0).astype(np.int64)        # source half per node (fixed)
    # per-node in-edge counts split by source half
    dest = col
    sh = half[row]
    cnt0 = np.bincount(dest[sh == 0], minlength=N)
    cnt1 = np.bincount(dest[sh == 1], minlength=N)

    # per-core packing: half-0 nodes -> blocks 0..H0B-1, half-1 -> rest
    newlocal = np.empty(N, np.int64)
    nvalid = np.zeros((C, TPC), np.int64)
    for c in range(C):
        base = c * NSH
        for h, (b0, nb) in enumerate(((0, H0B), (H0B, TPC - H0B))):
            ids = base + np.where((loc[base : base + NSH] >= H0) == bool(h))[0]
            d0 = cnt0[ids].astype(np.float64)
            d1 = cnt1[ids].astype(np.float64)
            avg0 = max(d0.sum() / nb, 1.0)
            avg1 = max(d1.sum() / nb, 1.0)
            cap0 = max(1024.0, np.ceil(avg0 / P) * P)
            cap1 = max(1024.0, np.ceil(avg1 / P) * P)
            blk, cnt = _pack_half(d0, d1, nb, cap0, cap1)
            rank = np.zeros(len(ids), np.int64)
            c2 = np.zeros(nb, np.int64)
            for k in range(len(ids)):
                rank[k] = c2[blk[k]]
                c2[blk[k]] += 1
            newlocal[ids] = (b0 + blk) * P + rank
            nvalid[c, b0 : b0 + nb] = c2
    assert nvalid.max() <= P

    # source table rows (per half)
    slot_of = newlocal  # node -> slot in its core's shard [0, NPAD)
    src_core = row // NSH
    src_slot = slot_of[row]
    src_half = (src_slot >= H0).astype(np.int64)
    src_t0 = src_core * cfg.H0 + src_slot
    src_t1 = src_core * cfg.H1 + (src_slot - H0)

    dst_core = col // NSH
    dst_slot = slot_of[col]
    dst_blk = dst_slot // P
    dst_rank = dst_slot % P

    # per (core, half): per-block edge lists sorted by src row
    cnts = np.zeros((C, 2, TPC), np.int64)
    per = {}
    for c in range(C):
        selc = dst_core == c
        for h in range(2):
            sel = selc & (src_half == h)
            srcs = (src_t0 if h == 0 else src_t1)[sel]
            blks = dst_blk[sel]
            rnks = dst_rank[sel]
            o = np.lexsort((srcs, blks))
            srcs, blks, rnks = srcs[o], blks[o], rnks[o]
            st = np.searchsorted(blks, np.arange(TPC))
            en = np.searchsorted(blks, np.arange(TPC) + 1)
            per[(c, h)] = (srcs, rnks, st, en)
            cnts[c, h] = en - st

    # shared tile budgets per (half, block)
    kb = np.maximum(1, -(-cnts.max(axis=0) // P))   # [2, TPC]
    NT = kb.sum(axis=1)                             # tiles per half stream
    off = np.zeros((2, TPC), np.int64)
    off[0, 1:] = np.cumsum(kb[0])[:-1]
    off[1, 1:] = np.cumsum(kb[1])[:-1]

    Wt = np.ascontiguousarray(np.transpose(np.asarray(W, np.float32), (0, 2, 1)))
    bT = np.ascontiguousarray(np.asarray(b, np.float32).T)
    iota = np.tile(np.arange(P, dtype=np.float32)[None, :], (P, 1))

    in_maps = []
    for c in range(C):
        idxs, ranks = [], []
        for h in range(2):
            idx = np.zeros(NT[h] * P, np.int64)
            rnk = np.full(NT[h] * P, SENTINEL, np.float32)
            srcs, rnks, st, en = per[(c, h)]
            for bidx in range(TPC):
                o0 = off[h, bidx] * P
                k = en[bidx] - st[bidx]
                idx[o0 : o0 + k] = srcs[st[bidx] : en[bidx]]
                rnk[o0 : o0 + k] = rnks[st[bidx] : en[bidx]]
            idxs.append(_wrap16(idx))
            ranks.append(np.ascontiguousarray(rnk.reshape(NT[h], P).T))
        ids = np.arange(c * NSH, (c + 1) * NSH)
        xin = np.zeros((NPAD, cfg.D), np.float32)
        xin[newlocal[ids]] = x[ids]
        degT = np.zeros((P, TPC), np.float32)
        degT[newlocal[ids] % P, newlocal[ids] // P] = deg[ids]
        maskv = (np.arange(P)[:, None] < nvalid[c][None, :]).astype(np.float32)
        in_maps.append(
            {
                "xin": xin,
                "wt": Wt,
                "bT": bT,
                "gamma": np.asarray(gamma, np.float32),
                "beta": np.asarray(beta, np.float32),
                "degT": degT,
                "maskv": maskv,
                "iota": iota,
                "idx0": idxs[0],
                "idx1": idxs[1],
                "rank0": ranks[0],
                "rank1": ranks[1],
            }
        )

    meta = dict(
        NT0=int(NT[0]),
        NT1=int(NT[1]),
        kb=kb,
        off=off,
        newlocal=newlocal,
    )
    return in_maps, meta


def _build(cfg, NT0, NT1, kb, off):
    N, D, L, C = cfg.N, cfg.D, cfg.L, cfg.C
    TPC, NPAD = cfg.TPC, cfg.NPAD
    KG = cfg.KG
    NTs = (NT0, NT1)

    nc = bacc.Bacc("TRN2", target_bir_lowering=False, debug=False, num_devices=C)

    xin = nc.dram_tensor("xin", [NPAD, D], F32, kind="ExternalInput")
    wt = nc.dram_tensor("wt", [L, D, D], F32, kind="ExternalInput")
    bT = nc.dram_tensor("bT", [D, L], F32, kind="ExternalInput")
    gamma_d = nc.dram_tensor("gamma", [L, D], F32, kind="ExternalInput")
    beta_d = nc.dram_tensor("beta", [L, D], F32, kind="ExternalInput")
    degT = nc.dram_tensor("degT", [P, TPC], F32, kind="ExternalInput")
    maskv_d = nc.dram_tensor("maskv", [P, TPC], F32, kind="ExternalInput")
    iota_d = nc.dram_tensor("iota", [P, P], F32, kind="ExternalInput")
    idx_d = [
        nc.dram_tensor(f"idx{h}", [P, NTs[h] * P // 16], mybir.dt.int16,
                       kind="ExternalInput")
        for h in range(2)
    ]
    rank_d = [
        nc.dram_tensor(f"rank{h}", [P, NTs[h]], F32, kind="ExternalInput")
        for h in range(2)
    ]
    KBMAX = int(kb.max())
    out_d = nc.dram_tensor("out", [NPAD, D], F32, kind="ExternalOutput")

    rg = [list(range(C))]

    with tile.TileContext(nc) as tc:
        with (
            tc.tile_pool(name="persist", bufs=1) as pp,
            tc.tile_pool(name="msgp", bufs=14) as msgp,
            tc.tile_pool(name="bigtmp", bufs=1) as btp,
            tc.tile_pool(name="sp", bufs=4) as sp,
            tc.tile_pool(name="work", bufs=4) as wp,
            tc.tile_pool(name="psblk", bufs=3, space="PSUM") as psblk,
            tc.tile_pool(name="psmisc", bufs=2, space="PSUM") as psmisc,
            tc.tile_pool(name="psbc", bufs=1, space="PSUM") as psbc,
            tc.tile_pool(name="psstat", bufs=2, space="PSUM") as psstat,
            tc.tile_pool(name="dram", bufs=1, space="DRAM") as dp,
        ):
            # ---- persistent loads ----
            x_sb = pp.tile([P, TPC, D], F32)
            nc.sync.dma_start(x_sb[:], xin[:].rearrange("(t p) f -> p t f", p=P))
            wt_sb = pp.tile([P, L, D], F32)
            for l in range(L):
                nc.sync.dma_start(wt_sb[:, l, :], wt[l, :, :])
            bT_sb = pp.tile([P, L], F32)
            nc.sync.dma_start(bT_sb[:], bT[:])
            gb_sb = pp.tile([1, 2 * L, D], F32)
            for l in range(L):
                nc.sync.dma_start(gb_sb[:, l, :], gamma_d[l : l + 1, :])
                nc.sync.dma_start(gb_sb[:, L + l, :], beta_d[l : l + 1, :])
            deg_sb = pp.tile([P, TPC], F32)
            nc.sync.dma_start(deg_sb[:], degT[:])
            maskv_sb = pp.tile([P, TPC], F32)
            nc.sync.dma_start(maskv_sb[:], maskv_d[:])
            iota_sb = pp.tile([P, P], F32)
            nc.sync.dma_start(iota_sb[:], iota_d[:])
            idx0_sb = pp.tile([P, NTs[0] * P // 16], mybir.dt.int16)
            idx1_sb = pp.tile([P, NTs[1] * P // 16], mybir.dt.int16)
            rank0_sb = pp.tile([P, NTs[0]], F32)
            rank1_sb = pp.tile([P, NTs[1]], F32)
            idx_sb = [idx0_sb, idx1_sb]
            rank_sb = [rank0_sb, rank1_sb]
            for h in range(2):
                nc.sync.dma_start(idx_sb[h][:], idx_d[h][:])
                nc.sync.dma_start(rank_sb[h][:], rank_d[h][:])
            ident = pp.tile([P, P], F32)
            make_identity(nc, ident[:])
            ones1 = pp.tile([1, P], F32)
            nc.vector.memset(ones1[:], 1.0)

            # dinv = (deg > 0) / sqrt(max(deg, 1))
            dinv_sb = pp.tile([P, TPC], F32)
            t_a = wp.tile([P, TPC], F32, tag="dinv")
            nc.vector.tensor_scalar_max(t_a[:], deg_sb[:], 1.0)
            t_b = wp.tile([P, TPC], F32, tag="dinv")
            nc.vector.reciprocal(t_b[:], t_a[:])
            t_c = wp.tile([P, TPC], F32, tag="dinv")
            nc.scalar.sqrt(t_c[:], t_b[:])
            t_d = wp.tile([P, TPC], F32, tag="dinv")
            nc.vector.tensor_scalar(t_d[:], deg_sb[:], 0.0, None, ALU.is_gt)
            nc.vector.tensor_tensor(dinv_sb[:], t_c[:], t_d[:], ALU.mult)

            agg_sb = pp.tile([P, TPC, D], F32)
            hs_sb = pp.tile([P, TPC, D], BF16)

            # DRAM collective buffers
            sh0_dr = dp.tile([cfg.H0, D], BF16)
            sh1_dr = dp.tile([cfg.H1, D], BF16)
            tab0_dr = dp.tile([cfg.T0ROWS, D], BF16)
            tab1_dr = dp.tile([cfg.T1ROWS, D], BF16)
            sh_dr = [sh0_dr, sh1_dr]
            tab_dr = [tab0_dr, tab1_dr]
            stats_in = dp.tile([1, 2 * D], F32)
            stats_out = dp.tile([1, 2 * D], F32)

            for l in range(L):
                # ---- hs = dinv * (x @ W^T + b), bf16; AG in two halves ----
                for t in range(TPC):
                    xT_ps = psmisc.tile([P, P], F32, tag="ps")
                    nc.tensor.transpose(xT_ps[:], x_sb[:, t, :], ident[:])
                    xT = wp.tile([P, P], F32, tag="xT")
                    nc.vector.tensor_copy(xT[:], xT_ps[:])
                    hT_ps = psmisc.tile([P, P], F32, tag="ps")
                    nc.tensor.matmul(
                        out=hT_ps[:], lhsT=wt_sb[:, l, :], rhs=xT[:],
                        start=True, stop=True,
                    )
                    hb = wp.tile([P, P], F32, tag="hb")
                    nc.scalar.activation(
                        hb[:], hT_ps[:], AF.Identity, bias=bT_sb[:, l : l + 1]
                    )
                    h_rm_ps = psmisc.tile([P, P], F32, tag="ps")
                    nc.tensor.transpose(h_rm_ps[:], hb[:], ident[:])
                    nc.scalar.activation(
                        hs_sb[:, t, :], h_rm_ps[:], AF.Identity,
                        scale=dinv_sb[:, t : t + 1],
                    )
                    if t == cfg.H0B - 1:
                        nc.sync.dma_start(
                            sh_dr[0][:].rearrange("(t p) f -> p t f", p=P),
                            hs_sb[:, : cfg.H0B, :],
                        )
                        nc.gpsimd.collective_compute(
                            "AllGather", ALU.bypass,
                            ins=[sh_dr[0].opt()], outs=[tab_dr[0].opt()],
                            replica_groups=rg,
                        )
                nc.sync.dma_start(
                    sh_dr[1][:].rearrange("(t p) f -> p t f", p=P),
                    hs_sb[:, cfg.H0B :, :],
                )
                nc.gpsimd.collective_compute(
                    "AllGather", ALU.bypass,
                    ins=[sh_dr[1].opt()], outs=[tab_dr[1].opt()],
                    replica_groups=rg,
                )

                # ---- gather + one-hot matmul aggregation, half by half ----
                for h in range(2):
                    NT = NTs[h]
                    slot_of = {}
                    next_tile = 0

                    def _issue_gathers(upto, h=h, NT=NT):
                        nonlocal next_tile
                        while next_tile < min(upto, NT):
                            g0 = next_tile
                            g1 = min(g0 + KG, NT)
                            mt = msgp.tile([P, KG, D], BF16, tag="msg")
                            for i in range(g1 - g0):
                                slot_of[g0 + i] = (mt, i)
                            nc.gpsimd.dma_gather(
                                mt[:, : g1 - g0, :],
                                tab_dr[h][:],
                                idx_sb[h][:, g0 * 8 : g1 * 8],
                                (g1 - g0) * P, (g1 - g0) * P, D,
                            )
                            next_tile = g1

                    if h == 1:
                        stA_ps = psstat.tile([1, P], F32, tag="st")
                        stB_ps = psstat.tile([1, P], F32, tag="st")
                    for bidx in range(TPC):
                        t0 = int(off[h, bidx])
                        nt = int(kb[h, bidx])
                        _issue_gathers(t0 + nt)
                        ps_b = psblk.tile([P, P], F32, tag="blk")
                        s_t = sp.tile([P, KBMAX, P], BF16, tag="s")
                        nc.vector.tensor_tensor(
                            s_t[:, :nt, :],
                            iota_sb[:].unsqueeze(1).to_broadcast([P, nt, P]),
                            rank_sb[h][:, t0 : t0 + nt]
                            .unsqueeze(2).to_broadcast([P, nt, P]),
                            ALU.is_equal,
                        )
                        for j in range(nt):
                            mt, sl = slot_of[t0 + j]
                            nc.tensor.matmul(
                                out=ps_b[:], lhsT=s_t[:, j, :], rhs=mt[:, sl, :],
                                start=(j == 0), stop=(j == nt - 1),
                            )
                        if h == 0:
                            nc.scalar.activation(
                                agg_sb[:, bidx, :], ps_b[:], AF.Identity
                            )
                        else:
                            nc.vector.tensor_tensor(
                                agg_sb[:, bidx, :], agg_sb[:, bidx, :],
                                ps_b[:], ALU.add,
                            )
                            nc.scalar.activation(
                                agg_sb[:, bidx, :], agg_sb[:, bidx, :],
                                AF.Identity, scale=dinv_sb[:, bidx : bidx + 1],
                            )
                            nc.tensor.matmul(
                                out=stA_ps[:],
                                lhsT=maskv_sb[:, bidx : bidx + 1],
                                rhs=agg_sb[:, bidx, :],
                                start=(bidx == 0), stop=(bidx == TPC - 1),
                                skip_group_check=True,
                            )
                            aggsq = wp.tile([P, P], F32, tag="aggsq")
                            nc.scalar.square(aggsq[:], agg_sb[:, bidx, :])
                            nc.tensor.matmul(
                                out=stB_ps[:],
                                lhsT=maskv_sb[:, bidx : bidx + 1],
                                rhs=aggsq[:],
                                start=(bidx == 0), stop=(bidx == TPC - 1),
                                skip_group_check=True,
                            )

                st_sb = wp.tile([1, 2, P], F32, tag="st")
                nc.vector.tensor_copy(st_sb[:, 0, :], stA_ps[:])
                nc.vector.tensor_copy(st_sb[:, 1, :], stB_ps[:])
                nc.sync.dma_start(stats_in[:], st_sb[:])
                nc.gpsimd.collective_compute(
                    "AllReduce", ALU.add,
                    ins=[stats_in.opt()], outs=[stats_out.opt()],
                    replica_groups=rg,
                )
                stg = wp.tile([1, 2, P], F32, tag="st")
                nc.sync.dma_start(stg[:], stats_out[:])

                # ---- scale/shift vectors on partition 0 ----
                vec = wp.tile([1, 8, P], F32, tag="vec")
                MU, MSQ, VAR, RSTD, SC, SH, T0, T1 = range(8)
                inv_n = 1.0 / float(N)
                nc.vector.tensor_scalar_mul(vec[:, MU, :], stg[:, 0, :], inv_n)
                nc.vector.tensor_scalar_mul(vec[:, MSQ, :], stg[:, 1, :], inv_n)
                nc.vector.tensor_tensor(
                    vec[:, T0, :], vec[:, MU, :], vec[:, MU, :], ALU.mult
                )
                nc.vector.tensor_tensor(
                    vec[:, VAR, :], vec[:, MSQ, :], vec[:, T0, :], ALU.subtract
                )
                nc.vector.tensor_scalar_add(vec[:, T1, :], vec[:, VAR, :], cfg.BN_EPS)
                nc.vector.reciprocal(vec[:, T0, :], vec[:, T1, :])
                nc.scalar.sqrt(vec[:, RSTD, :], vec[:, T0, :])
                nc.vector.tensor_tensor(
                    vec[:, SC, :], gb_sb[:, l, :], vec[:, RSTD, :], ALU.mult
                )
                nc.vector.tensor_tensor(
                    vec[:, T0, :], vec[:, MU, :], vec[:, SC, :], ALU.mult
                )
                nc.vector.tensor_tensor(
                    vec[:, SH, :], gb_sb[:, L + l, :], vec[:, T0, :], ALU.subtract
                )
                bc_ps = psbc.tile([P, 2 * P], F32, tag="bc")
                nc.tensor.matmul(
                    out=bc_ps[:], lhsT=ones1[:], rhs=vec[:, SC : SH + 1, :],
                    start=True, stop=True,
                )
                screp = wp.tile([P, 2, P], F32, tag="screp")
                nc.vector.tensor_copy(screp[:], bc_ps[:])

                # ---- BN apply + relu + residual ----
                t1 = btp.tile([P, TPC, D], F32, tag="t1")
                nc.vector.tensor_tensor(
                    t1[:], agg_sb[:],
                    screp[:, 0:1, :].to_broadcast([P, TPC, D]), ALU.mult,
                )
                nc.vector.tensor_tensor(
                    t1[:], t1[:],
                    screp[:, 1:2, :].to_broadcast([P, TPC, D]), ALU.add,
                )
                nc.scalar.activation(t1[:], t1[:], AF.Relu)
                nc.vector.tensor_tensor(x_sb[:], x_sb[:], t1[:], ALU.add)

            nc.sync.dma_start(out_d[:].rearrange("(t p) f -> p t f", p=P), x_sb[:])

    nc.compile()
    return nc


_CACHE = {}


def _get_nc(cfg, NT0, NT1, kb, off):
    key = (cfg.N, cfg.E, cfg.L, cfg.C, cfg.KG, NT0, NT1,
           tuple(kb.ravel()), tuple(off.ravel()))
    if key not in _CACHE:
        _CACHE[key] = _build(cfg, NT0, NT1, kb, off)
    return _CACHE[key]


def run(cfg, inputs, trace=False):
    in_maps, meta = _preprocess(cfg, **inputs)
    nc = _get_nc(cfg, meta["NT0"], meta["NT1"], meta["kb"], meta["off"])
    res = run_bass_kernel_spmd(nc, in_maps, core_ids=list(range(cfg.C)), trace=trace)
    newlocal = meta["newlocal"]
    xfull = np.empty((cfg.N, cfg.D), np.float32)
    for c in range(cfg.C):
        ids = np.arange(c * cfg.NSH, (c + 1) * cfg.NSH)
        xfull[ids] = res.results[c]["out"][newlocal[ids]]
    return xfull, res


def kernel(x, edge_index, W, b, gamma, beta):
    cfg = Cfg(N=50000, E=800000, D=128, L=3, C=8, kg=8)
    out, _ = run(
        cfg, dict(x=x, edge_index=edge_index, W=W, b=b, gamma=gamma, beta=beta)
    )
    return out
